# revision 5
# baseline (speedup 1.0000x reference)
"""Trainium2 Bass kernel for the conv-encoder + Graves-attention GRU decoder.

Strategy (8 NeuronCores, data-parallel over batch, 4 rows/core, no collectives):
  - Host: conv encoder via im2col BLAS matmul (~1% of FLOPs), then per-core
    input staging in device-friendly transposed layouts.
  - Device phase 1: P[(b,l), g] = ctx[(b,l), f] @ W_ih[g, f].T  -- the 50MB
    W_ih streams through SBUF exactly once (float32r matmuls, DMA-bound).
    This moves all heavy FLOPs out of the sequential scan, because
    attended @ W_ih.T == sum_l w[b,l] * P[b,l,:]  (attention is linear).
  - Device phase 2: 64-step GRU scan, everything resident in SBUF.
    Layouts keep the large dim (gate/hidden) on partitions, batch on the
    free dim, so elementwise ops are cheap and matmuls are well-formed.
"""

import numpy as np

import concourse.bass as bass
import concourse.mybir as mybir
from concourse.tile import TileContext
from concourse.vector_clock import ScopedClock
from concourse.bass_utils import run_bass_kernel_spmd

# Problem shape (hardcoded per contract)
B, CIN, NBINS, NFRAMES = 32, 1, 128, 256
NFILT, DDEC, NCLS, TOUT = 64, 512, 17, 64
HC, WC = 64, 128            # conv output spatial dims (L, and width)
F = NFILT * WC              # 8192 encoder feature dim
G = 3 * DDEC                # 1536 stacked GRU gates [r, z, n]
NCORES = 8
BL = B // NCORES            # 4 batch rows per core
NKH = DDEC // 128           # 4 hidden-dim partition chunks
NFC = F // 128              # 64 feature-dim partition chunks
NGT = G // 128              # 12 gate-dim partition tiles

f32 = mybir.dt.float32
f32r = mybir.dt.float32r

AF = mybir.ActivationFunctionType
ALU = mybir.AluOpType


def _split_multi_waits(nc):
    """This container's walrus allows only ONE sync-wait per instruction.
    Hoist extra waits onto injected same-engine NOPs placed just before."""
    k = 0
    for f in nc.m.functions:
        for bb in f.blocks:
            insts = list(bb.instructions)
            out = []
            for inst in insts:
                si = getattr(inst, "sync_info", None)
                waits = list(si.on_wait) if (si is not None and si.on_wait) else []
                if len(waits) > 1:
                    for w in waits[:-1]:
                        nop = mybir.InstNoOp(
                            name=f"wsplit-{k}", ins=[], outs=[],
                            sync_info=mybir.SyncInfo(on_wait=[w], on_update=[]))
                        nop.engine = inst.engine
                        out.append(nop)
                        k += 1
                    si.on_wait = waits[-1:]
                out.append(inst)
            bb.instructions = out
    return k


def _build_nc():
    nc = bass.Bass("TRN2")

    # ---- DRAM I/O (per-core) ----
    ctxT_d = nc.dram_tensor("ctxT", [128, NFC, BL * HC], f32r, kind="ExternalInput")
    wihT_d = nc.dram_tensor("wihT", [NFC, 128, G], f32r, kind="ExternalInput")
    whhT_d = nc.dram_tensor("whhT", [128, NKH, G], f32, kind="ExternalInput")
    kwT_d = nc.dram_tensor("kwT", [128, NKH], f32, kind="ExternalInput")
    bwT_d = nc.dram_tensor("bwT", [128, NKH], f32, kind="ExternalInput")
    outwT_d = nc.dram_tensor("outwT", [128, NKH, NCLS], f32, kind="ExternalInput")
    outb_d = nc.dram_tensor("outb", [NCLS, 1], f32, kind="ExternalInput")
    u2_d = nc.dram_tensor("u2", [128, BL], f32, kind="ExternalInput")
    ones_d = nc.dram_tensor("ones", [1, 128], f32, kind="ExternalInput")

    outs_d = nc.dram_tensor("outs_d", [NCLS, TOUT, BL], f32, kind="ExternalOutput")
    ws_d = nc.dram_tensor("ws_d", [HC, TOUT, BL], f32, kind="ExternalOutput")

    NPAIR = BL // 2  # 2 partition-pair tiles of (b, l)

    with TileContext(nc) as tc:
        with (
            tc.tile_pool(name="const", bufs=1) as cpool,
            tc.tile_pool(name="state", bufs=1) as spool_s,
            tc.tile_pool(name="wih", bufs=3) as wpool,
            tc.tile_pool(name="scratch", bufs=2) as xpool,
        ):
            # ---- Resident tiles ----
            ctxT = cpool.tile([128, NFC * BL * HC], f32r, tag="ctxT")
            whhT = cpool.tile([128, NKH * G], f32, tag="whhT")
            kwT = cpool.tile([128, NKH], f32, tag="kwT")
            bwT = cpool.tile([128, NKH], f32, tag="bwT")
            outwT = cpool.tile([128, NKH * NCLS], f32, tag="outwT")
            outb = cpool.tile([NCLS, 1], f32, tag="outb")
            u2 = cpool.tile([128, BL], f32, tag="u2")
            ones = cpool.tile([1, 128], f32, tag="ones")
            P_sb = [cpool.tile([128, G], f32, tag=f"P{m}", name=f"P{m}")
                    for m in range(NPAIR)]

            outs_sb = spool_s.tile([NCLS, TOUT * BL], f32, tag="outs")
            ws_sb = spool_s.tile([HC, TOUT * BL], f32, tag="ws")
            hT = spool_s.tile([128, NKH * BL], f32, tag="hT")
            kap = spool_s.tile([1, BL], f32, tag="kap")

            # ---- Load constants / state ----
            CH = 8  # ctxT DMA chunks
            for i in range(CH):
                s = NFC // CH
                nc.sync.dma_start(
                    ctxT[:, i * s * BL * HC:(i + 1) * s * BL * HC],
                    ctxT_d[:, i * s:(i + 1) * s, :],
                )
            nc.sync.dma_start(whhT[:], whhT_d[:])
            nc.sync.dma_start(kwT[:], kwT_d[:])
            nc.sync.dma_start(bwT[:], bwT_d[:])
            nc.sync.dma_start(outwT[:], outwT_d[:])
            nc.sync.dma_start(outb[:], outb_d[:])
            nc.sync.dma_start(u2[:], u2_d[:])
            nc.sync.dma_start(ones[:], ones_d[:])
            nc.vector.memset(hT[:], 0.0)
            nc.vector.memset(kap[:], 0.0)

            # ---- Phase 1: P[(b,l), g] = ctx @ W_ih.T, streaming W_ih ----
            ph1 = tc.tile_pool(name="psum_p", bufs=1, space="PSUM")
            pppool = ph1.__enter__()
            P_ps = [pppool.tile([128, G], f32, tag=f"Pps{m}", name=f"Pps{m}")
                    for m in range(NPAIR)]
            NNC = G // 512  # 3 moving chunks of 512
            for fc in range(NFC):
                wt = wpool.tile([128, G], f32r, tag="wih")
                nc.sync.dma_start(wt[:], wihT_d[fc])
                for m in range(NPAIR):
                    lhsT = ctxT[:, fc * BL * HC + m * 128: fc * BL * HC + (m + 1) * 128]
                    for j in range(NNC):
                        nc.tensor.matmul(
                            P_ps[m][:, j * 512:(j + 1) * 512],
                            lhsT,
                            wt[:, j * 512:(j + 1) * 512],
                            start=(fc == 0),
                            stop=(fc == NFC - 1),
                        )
            for m in range(NPAIR):
                nc.scalar.activation(P_sb[m][:], P_ps[m][:], AF.Copy)
            ph1.__exit__(None, None, None)

            # ---- Phase 2: the 64-step scan ----
            ph2a = tc.tile_pool(name="psum_a", bufs=1, space="PSUM")
            pspool = ph2a.__enter__()
            ph2g = tc.tile_pool(name="psum_g", bufs=2, space="PSUM")
            pgpool = ph2g.__enter__()
            for t in range(TOUT):
                # kappa/beta projections:  (1,BL) each
                ps_k = pspool.tile([1, BL], f32, tag="ps_k")
                ps_b = pspool.tile([1, BL], f32, tag="ps_b")
                for kh in range(NKH):
                    rhs = hT[:, kh * BL:(kh + 1) * BL]
                    nc.tensor.matmul(ps_k[:], kwT[:, kh:kh + 1], rhs,
                                     start=(kh == 0), stop=(kh == NKH - 1))
                    nc.tensor.matmul(ps_b[:], bwT[:, kh:kh + 1], rhs,
                                     start=(kh == 0), stop=(kh == NKH - 1))
                # kappa += dk  (persistent state, partition 0)
                nc.vector.tensor_tensor(kap[:], kap[:], ps_k[:], ALU.add)
                bl_sb = xpool.tile([1, BL], f32, tag="bl")
                nc.scalar.activation(bl_sb[:], ps_b[:], AF.Copy)
                # broadcast kappa,betalog across partitions via ones-matmul
                ps_bc = pspool.tile([128, 2 * BL], f32, tag="ps_bc")
                nc.tensor.matmul(ps_bc[:, 0:BL], ones[:], kap[:],
                                 start=True, stop=True)
                nc.tensor.matmul(ps_bc[:, BL:2 * BL], ones[:], bl_sb[:],
                                 start=True, stop=True)
                # w2[p, b] = exp(-exp(blog_b) * (kap_b - u[p%64])^2), (128, BL)
                d_sb = xpool.tile([128, BL], f32, tag="d")
                nc.vector.tensor_tensor(d_sb[:], ps_bc[:, 0:BL], u2[:], ALU.subtract)
                d2_sb = xpool.tile([128, BL], f32, tag="d2")
                nc.vector.tensor_tensor(d2_sb[:], d_sb[:], d_sb[:], ALU.mult)
                e_sb = xpool.tile([128, BL], f32, tag="e")
                nc.scalar.activation(e_sb[:], ps_bc[:, BL:2 * BL], AF.Exp)
                m_sb = xpool.tile([128, BL], f32, tag="m")
                nc.vector.tensor_tensor(m_sb[:], e_sb[:], d2_sb[:], ALU.mult)
                w2_sb = xpool.tile([128, BL], f32, tag="w2")
                nc.scalar.activation(w2_sb[:], m_sb[:], AF.Exp, scale=-1.0)
                # attention weights output (rows 0..63 are l=0..63)
                nc.vector.tensor_copy(
                    out=ws_sb[:, t * BL:(t + 1) * BL], in_=w2_sb[0:HC, :])

                # gates psum: [0:2*NKH*BL] r|z accum (gh+gx), then ghn, gxn
                NB4 = NKH * BL  # 16 cols per gate
                ps_g = pgpool.tile([128, 4 * NB4], f32, tag="ps_g")
                for gt in range(NGT):
                    gate = gt // NKH          # 0:r 1:z 2:n
                    sub = gt % NKH            # 128-row tile within the gate
                    # gh contributions
                    for kh in range(NKH):
                        lhsT = whhT[:, kh * G + gt * 128: kh * G + (gt + 1) * 128]
                        rhs = hT[:, kh * BL:(kh + 1) * BL]
                        if gate < 2:
                            out = ps_g[:, gate * NB4 + sub * BL: gate * NB4 + (sub + 1) * BL]
                            nc.tensor.matmul(out, lhsT, rhs,
                                             start=(kh == 0), stop=False)
                        else:
                            out = ps_g[:, 2 * NB4 + sub * BL: 2 * NB4 + (sub + 1) * BL]
                            nc.tensor.matmul(out, lhsT, rhs,
                                             start=(kh == 0), stop=(kh == NKH - 1))
                    # gx contributions
                    for b in range(BL):
                        mpair, half = b // 2, b % 2
                        lhsT = P_sb[mpair][64 * half:64 * half + 64,
                                           gt * 128:(gt + 1) * 128]
                        rhs = w2_sb[64 * half:64 * half + 64, b:b + 1]
                        if gate < 2:
                            out = ps_g[:, gate * NB4 + sub * BL + b:
                                       gate * NB4 + sub * BL + b + 1]
                            nc.tensor.matmul(out, lhsT, rhs,
                                             start=False, stop=(b == BL - 1))
                        else:
                            out = ps_g[:, 3 * NB4 + sub * BL + b:
                                       3 * NB4 + sub * BL + b + 1]
                            nc.tensor.matmul(out, lhsT, rhs,
                                             start=(b == 0), stop=(b == BL - 1))

                # gate math, all (128, 16)
                r_sb = xpool.tile([128, NB4], f32, tag="r")
                nc.scalar.activation(r_sb[:], ps_g[:, 0:NB4], AF.Sigmoid)
                z_sb = xpool.tile([128, NB4], f32, tag="z")
                nc.scalar.activation(z_sb[:], ps_g[:, NB4:2 * NB4], AF.Sigmoid)
                t1 = xpool.tile([128, NB4], f32, tag="t1")
                nc.vector.tensor_tensor(t1[:], r_sb[:], ps_g[:, 2 * NB4:3 * NB4],
                                        ALU.mult)
                t2 = xpool.tile([128, NB4], f32, tag="t2")
                nc.vector.tensor_tensor(t2[:], t1[:], ps_g[:, 3 * NB4:4 * NB4],
                                        ALU.add)
                n_sb = xpool.tile([128, NB4], f32, tag="n")
                nc.scalar.activation(n_sb[:], t2[:], AF.Tanh)
                t3 = xpool.tile([128, NB4], f32, tag="t3")
                nc.vector.tensor_tensor(t3[:], hT[:], n_sb[:], ALU.subtract)
                t4 = xpool.tile([128, NB4], f32, tag="t4")
                nc.vector.tensor_tensor(t4[:], z_sb[:], t3[:], ALU.mult)
                nc.vector.tensor_tensor(hT[:], n_sb[:], t4[:], ALU.add)

                # output projection (17, BL)
                ps_o = pspool.tile([NCLS, BL], f32, tag="ps_o")
                for kh in range(NKH):
                    nc.tensor.matmul(
                        ps_o[:],
                        outwT[:, kh * NCLS:(kh + 1) * NCLS],
                        hT[:, kh * BL:(kh + 1) * BL],
                        start=(kh == 0), stop=(kh == NKH - 1))
                nc.scalar.activation(outs_sb[:, t * BL:(t + 1) * BL], ps_o[:],
                                     AF.Identity, bias=outb[:])

            ph2g.__exit__(None, None, None)
            ph2a.__exit__(None, None, None)

            # ---- Store outputs ----
            nc.sync.dma_start(outs_d[:], outs_sb[:])
            nc.sync.dma_start(ws_d[:], ws_sb[:])

    _split_multi_waits(nc)
    return nc


_compiled_nc = None


def _host_prep(x, conv_w, kappa_w, beta_w, gru_w_ih, gru_w_hh, out_w, out_b):
    """Build per-core input dicts (host conv + transposed layouts)."""
    x = np.asarray(x, np.float32)
    conv_w = np.asarray(conv_w, np.float32)
    # conv via im2col + BLAS
    xpad = np.zeros((B, NBINS + 2, NFRAMES + 2), np.float32)
    xpad[:, 1:-1, 1:-1] = x[:, 0]
    patches = np.empty((B, HC, WC, 9), np.float32)
    for kh in range(3):
        for kw in range(3):
            patches[..., kh * 3 + kw] = xpad[:, kh:kh + 2 * HC:2, kw:kw + 2 * WC:2]
    conv = patches.reshape(-1, 9) @ conv_w.reshape(NFILT, 9).T.astype(np.float32)
    # ctx[b, l, f] with f = c*WC + wc ; conv rows are (b, hc, wc), cols c
    ctx = conv.reshape(B, HC, WC, NFILT).transpose(0, 1, 3, 2).reshape(B, HC, F)

    w_ih = np.asarray(gru_w_ih, np.float32)
    w_hh = np.asarray(gru_w_hh, np.float32)
    kappa_w = np.asarray(kappa_w, np.float32).reshape(DDEC)
    beta_w = np.asarray(beta_w, np.float32).reshape(DDEC)
    out_w = np.asarray(out_w, np.float32)
    out_b = np.asarray(out_b, np.float32)

    # shared (replicated) arrays
    wihT = np.ascontiguousarray(
        w_ih.T.reshape(NFC, 128, G))                       # [fc, p, g]
    whhT = np.ascontiguousarray(
        w_hh.T.reshape(NKH, 128, G).transpose(1, 0, 2))    # [p, kh, g]
    kwT = np.ascontiguousarray(kappa_w.reshape(NKH, 128).T)   # [p, kh]
    bwT = np.ascontiguousarray(beta_w.reshape(NKH, 128).T)
    outwT = np.ascontiguousarray(
        out_w.T.reshape(NKH, 128, NCLS).transpose(1, 0, 2))  # [p, kh, c]
    outb = np.ascontiguousarray(out_b.reshape(NCLS, 1))
    u2 = np.broadcast_to(
        (np.arange(128, dtype=np.float32) % HC)[:, None], (128, BL)).copy()
    ones = np.ones((1, 128), np.float32)

    in_maps = []
    for c in range(NCORES):
        cb = ctx[c * BL:(c + 1) * BL]                      # (BL, HC, F)
        # ctxT[p, fc, (b,l)] = ctx[b, l, fc*128+p]
        ctxT = np.ascontiguousarray(
            cb.reshape(BL * HC, NFC, 128).transpose(2, 1, 0))
        in_maps.append(dict(
            ctxT=ctxT, wihT=wihT, whhT=whhT, kwT=kwT, bwT=bwT,
            outwT=outwT, outb=outb, u2=u2, ones=ones,
        ))
    return in_maps


def kernel(x, output_len, conv_w, kappa_w, beta_w, gru_w_ih, gru_w_hh,
           out_w, out_b):
    global _compiled_nc
    assert int(output_len) == TOUT
    in_maps = _host_prep(x, conv_w, kappa_w, beta_w, gru_w_ih, gru_w_hh,
                         out_w, out_b)
    if _compiled_nc is None:
        _compiled_nc = _build_nc()
    res = run_bass_kernel_spmd(_compiled_nc, in_maps,
                               core_ids=list(range(NCORES)))
    outs = np.empty((B, TOUT, NCLS), np.float32)
    ws = np.empty((B, TOUT, HC), np.float32)
    for c in range(NCORES):
        outs[c * BL:(c + 1) * BL] = res.results[c]["outs_d"].transpose(2, 1, 0)
        ws[c * BL:(c + 1) * BL] = res.results[c]["ws_d"].transpose(2, 1, 0)
    return outs, ws
